# revision 9
# baseline (speedup 1.0000x reference)
"""Trainium2 Bass kernel for nn_EncoderLayer (gnn_message_passing).

Layer: node message-passing MLP + LN + FFN + LN, then edge-update MLP + LN.
Shapes hardcoded: B=4, N=2048, K=48, C=128.

Sharding: 8 cores, each owns 1024 (b, n) rows (half a batch); weights
replicated.  Neighbor gathers h_V[E_idx] are folded through the first
linear layer: host precomputes U = h_V @ W1c + b1 per batch, packs it
hi/lo-bf16 into 512-byte rows, and the device gathers rows channel-major
via dma_gather(transpose=True) at full DMA rate, accumulating into PSUM
with identity matmuls (hi + lo recovers ~fp32 precision).

Two launches: (1) node update -> h_V_new, (2) edge update (needs the
updated h_V of *all* nodes in a batch for its gather, so the new gather
table is built on host between launches).
"""

import numpy as np
import ml_dtypes
from contextlib import ExitStack

import concourse.bacc as bacc
import concourse.bass as bass
import concourse.tile as tile
import concourse.mybir as mybir
from concourse import library_config
from concourse.bass_utils import run_bass_kernel_spmd

F32 = mybir.dt.float32
F32R = mybir.dt.float32r
BF16 = mybir.dt.bfloat16
I16 = mybir.dt.int16
AF = mybir.ActivationFunctionType
ALU = mybir.AluOpType
AX = mybir.AxisListType

B, N, K, C = 4, 2048, 48, 128
NCORES = 8
ROWS = B * N // NCORES          # 1024 rows per core
EDG = ROWS * K                  # 49152 edges per core
CH = 384                        # edges per chunk = 8 nodes * 48
CN = CH // K                    # nodes per chunk = 8
NCHUNK = EDG // CH              # 128
NTILE = CH // 128               # 3 transpose tiles per chunk
SCALE = 30.0
EPS = 1e-5


def _bf(x):
    return x.astype(ml_dtypes.bfloat16)


def _hi_lo_pack(x):
    """fp32 [n, C] -> [n, 2C] bf16 rows [hi | lo]."""
    hi = _bf(x)
    lo = _bf(x - hi.astype(np.float32))
    return np.ascontiguousarray(np.concatenate([hi, lo], axis=-1))


def _wrap_idx(flat):
    """[E] int -> [128, E//16] int16 (idx i at [i%16, i//16], x8 replicated)."""
    w = flat.reshape(-1, 16).T.astype(np.int16)
    return np.ascontiguousarray(np.tile(w, (8, 1)))


def _kbcast(ap2d, reps):
    """[p, n] AP -> [p, n, reps] AP with step-0 innermost dim."""
    return bass.AP(
        tensor=ap2d.tensor,
        offset=ap2d.offset,
        ap=[ap2d.ap[0], ap2d.ap[1], [0, reps]],
    )


def _pbcast(ap, parts=128):
    """[1, ...] AP -> same free dims broadcast across `parts` partitions."""
    return bass.AP(
        tensor=ap.tensor, offset=ap.offset,
        ap=[[0, parts]] + [list(d) for d in ap.ap[1:]],
    )


def _ln_rows(nc, small, x_ap, eps_ap, gb=None, bb=None, rowscale=None):
    """LayerNorm over the free dim (C) of x_ap ([128, C], row-major),
    in place.  gb/bb: optional [128, C] gamma/beta broadcast APs.
    rowscale: optional [128, 1] per-row AP multiplied in at the end."""
    st = small.tile([128, 6], F32, tag="ln_st", bufs=2)
    nc.vector.bn_stats(st[:], x_ap)
    mv = small.tile([128, 2], F32, tag="ln_mv", bufs=2)
    nc.vector.bn_aggr(mv[:], st[:])
    sd = small.tile([128, 1], F32, tag="ln_sd", bufs=2)
    nc.scalar.activation(sd[:], mv[:, 1:2], AF.Sqrt, bias=eps_ap)
    rs = small.tile([128, 1], F32, tag="ln_rs", bufs=2)
    nc.vector.reciprocal(rs[:], sd[:])
    nc.vector.tensor_scalar(
        x_ap, x_ap, mv[:, 0:1], rs[:], ALU.subtract, ALU.mult)
    if gb is not None:
        nc.vector.tensor_tensor(x_ap, x_ap, gb, ALU.mult)
    if bb is not None:
        nc.vector.tensor_tensor(x_ap, x_ap, bb, ALU.add)
    if rowscale is not None:
        nc.vector.tensor_scalar_mul(x_ap, x_ap, rowscale)


def _build_core(eph, *, trivial_gb, trivial_mask_att, debug=False,
                nchunk=NCHUNK, act=AF.Gelu):
    """Build the Bass module for one phase.

    eph=False: node-update phase.  eph=True: edge-update phase.
    trivial_gb: this phase's LN gammas/betas are (1, 0) -> skip apply.
    trivial_mask_att: mask_attend is all-ones -> skip per-edge multiply.
    """
    nc = bacc.Bacc("TRN2", target_bir_lowering=False, debug=debug)
    edg = nchunk * CH

    hE = nc.dram_tensor("hE", [edg, C], F32, kind="ExternalInput")
    idx = nc.dram_tensor("idx", [128, edg // 16], I16, kind="ExternalInput")
    utab = nc.dram_tensor("utab", [N, 2 * C], BF16, kind="ExternalInput")
    pT = nc.dram_tensor("pT", [C, ROWS], F32, kind="ExternalInput")
    wb = nc.dram_tensor("wb", [C, C], F32, kind="ExternalInput")
    w2 = nc.dram_tensor("w2", [C, C], F32, kind="ExternalInput")
    w3 = nc.dram_tensor("w3", [C, C], F32, kind="ExternalInput")
    b2v = nc.dram_tensor("b2v", [C, 1], F32, kind="ExternalInput")
    ident = nc.dram_tensor("ident", [128, 128], F32, kind="ExternalInput")
    identb = nc.dram_tensor("identb", [128, 128], BF16, kind="ExternalInput")
    mattf = None
    if not trivial_mask_att:
        mattf = nc.dram_tensor("matt", [1, edg], F32, kind="ExternalInput")
    if not eph:
        hV = nc.dram_tensor("hV", [ROWS, C], F32, kind="ExternalInput")
        b3v = nc.dram_tensor("b3v", [1, C], F32, kind="ExternalInput")
        msum = nc.dram_tensor("msum", [1, ROWS], F32, kind="ExternalInput")
        wfi = nc.dram_tensor("wfi", [C, 4 * C], F32, kind="ExternalInput")
        wfo = nc.dram_tensor("wfo", [4 * C, C], F32, kind="ExternalInput")
        bfi = nc.dram_tensor("bfi", [128, 4], F32, kind="ExternalInput")
        bfo = nc.dram_tensor("bfo", [C, 1], F32, kind="ExternalInput")
        maskv = nc.dram_tensor("maskv", [128, ROWS // 128], F32,
                               kind="ExternalInput")
        out = nc.dram_tensor("out", [ROWS, C], F32, kind="ExternalOutput")
    else:
        b3v = nc.dram_tensor("b3v", [C, 1], F32, kind="ExternalInput")
        out = nc.dram_tensor("out", [edg, C], F32, kind="ExternalOutput")
    gbeta = None
    if not trivial_gb:
        gcols = 2 * C if eph else 4 * C
        gbeta = nc.dram_tensor("gbeta", [128, gcols], F32,
                               kind="ExternalInput")

    with tile.TileContext(nc) as tc, ExitStack() as ctx:
        nc.gpsimd.load_library(library_config.mlp)

        ones = ctx.enter_context(tc.tile_pool(name="ones", bufs=1))
        small = ctx.enter_context(tc.tile_pool(name="small", bufs=2))
        work = ctx.enter_context(tc.tile_pool(name="work", bufs=3))

        # ---- constants / weights in SBUF ----
        idn = ones.tile([128, 128], F32)
        nc.sync.dma_start(idn[:], ident[:])
        idnr = ones.tile([128, 128], F32R, tag="idnr")
        nc.gpsimd.dma_start(idnr[:], ident[:])
        eps_t = ones.tile([128, 1], F32, tag="eps_t")
        nc.vector.memset(eps_t[:], EPS)
        idnb = ones.tile([128, 128], BF16)
        nc.sync.dma_start(idnb[:], identb[:])
        wb_s = ones.tile([C, C], F32R)
        nc.gpsimd.dma_start(wb_s[:], wb[:])
        w2_s = ones.tile([C, C], F32R)
        nc.gpsimd.dma_start(w2_s[:], w2[:])
        if eph:
            w3_s = ones.tile([C, C], F32R)
            nc.gpsimd.dma_start(w3_s[:], w3[:])
        else:
            w3_s = ones.tile([C, C], F32)
            nc.sync.dma_start(w3_s[:], w3[:])
        b2_s = ones.tile([C, 1], F32)
        nc.sync.dma_start(b2_s[:], b2v[:])
        pT_s = ones.tile([C, ROWS], F32R)
        nc.gpsimd.dma_start(pT_s[:], pT[:])
        idx_s = ones.tile([128, edg // 16], I16)
        nc.sync.dma_start(idx_s[:], idx[:])
        gb1 = bb1 = gb2 = bb2 = None
        if gbeta is not None:
            g1t = ones.tile([128, 128], F32, tag="g1t")
            b1t = ones.tile([128, 128], F32, tag="b1t")
            nc.sync.dma_start(g1t[:], gbeta[:, 0:128])
            nc.sync.dma_start(b1t[:], gbeta[:, 128:256])
            gb1, bb1 = g1t[:], b1t[:]
            if not eph:
                g2t = ones.tile([128, 128], F32, tag="g2t")
                b2t = ones.tile([128, 128], F32, tag="b2t")
                nc.sync.dma_start(g2t[:], gbeta[:, 256:384])
                nc.sync.dma_start(b2t[:], gbeta[:, 384:512])
                gb2, bb2 = g2t[:], b2t[:]
        if eph:
            b3_s = ones.tile([C, 1], F32)
            nc.sync.dma_start(b3_s[:], b3v[:])
        else:
            b3_s = ones.tile([1, C], F32)
            nc.sync.dma_start(b3_s[:], b3v[:])
            msum_s = ones.tile([1, ROWS], F32)
            nc.sync.dma_start(msum_s[:], msum[:])
            hV_s = ones.tile([128, ROWS // 128, C], F32)
            nc.sync.dma_start(
                hV_s[:], hV[:].rearrange("(t p) c -> p t c", p=128))
            s2_s = ones.tile([C, ROWS], F32)
            if nchunk != NCHUNK:
                nc.vector.memset(s2_s[:], 0.0)

        # ---- main per-chunk pipeline over edges ----
        with tc.tile_pool(name="psA", bufs=2, space="PSUM") as psA:
            for ci in range(nchunk):
                e0 = ci * CH
                he_nat = work.tile([128, NTILE, C], F32, tag="he_nat")
                nc.sync.dma_start(
                    he_nat[:],
                    hE[e0:e0 + CH, :].rearrange("(t p) c -> p t c", p=128))
                heT_p = psA.tile([128, CH], F32, tag="heT_p")
                for t in range(NTILE):
                    nc.tensor.transpose(
                        heT_p[:, t * 128:(t + 1) * 128],
                        he_nat[:, t, :], idn[:])
                heT = work.tile([128, CH], F32R, tag="heT")
                nc.scalar.copy(heT[:], heT_p[:])
                gat = work.tile([128, 2, CH], BF16, tag="gat")
                nc.gpsimd.dma_gather(
                    gat[:], utab[:], idx_s[:, e0 // 16:(e0 + CH) // 16],
                    CH, CH, 2 * C, transpose=True)
                # pre1 = Wb.T @ heT + P[:, n]-broadcast + U_hi + U_lo
                pre1 = psA.tile([128, CH], F32, tag="pre1")
                nc.tensor.matmul(pre1[:], wb_s[:], heT[:],
                                 start=True, stop=False)
                nc.tensor.matmul(
                    pre1[:], idnr[:],
                    _kbcast(pT_s[:, ci * CN:(ci + 1) * CN], K),
                    start=False, stop=False)
                nc.tensor.matmul(pre1[:], idnb[:], gat[:, 0, :],
                                 start=False, stop=False)
                nc.tensor.matmul(pre1[:], idnb[:], gat[:, 1, :],
                                 start=False, stop=True)
                h1 = work.tile([128, CH], F32R, tag="h1")
                nc.scalar.activation(h1[:], pre1[:], act)
                p2 = psA.tile([128, CH], F32, tag="p2")
                nc.tensor.matmul(p2[:], w2_s[:], h1[:], start=True, stop=True)
                h2 = work.tile([128, CH], F32 if not eph else F32R, tag="h2")
                nc.scalar.activation(h2[:], p2[:], act, bias=b2_s[:])
                if mattf is not None:
                    mb = work.tile([128, CH], F32, tag="mb")
                    nc.sync.dma_start(mb[:], _pbcast(mattf[:, e0:e0 + CH]))
                    nc.vector.tensor_tensor(h2[:], h2[:], mb[:], ALU.mult)

                if not eph:
                    nc.vector.tensor_reduce(
                        s2_s[:, ci * CN:(ci + 1) * CN],
                        h2[:].rearrange("p (n k) -> p n k", k=K),
                        AX.X, ALU.add)
                else:
                    p3 = psA.tile([128, CH], F32, tag="p3", bufs=1)
                    nc.tensor.matmul(p3[:], w3_s[:], h2[:],
                                     start=True, stop=True)
                    m3 = work.tile([128, CH], F32R, tag="m3")
                    nc.scalar.activation(m3[:], p3[:], AF.Identity,
                                         bias=b3_s[:])
                    em = psA.tile([128, NTILE, 128], F32, tag="em", bufs=1)
                    for t in range(NTILE):
                        nc.tensor.transpose(
                            em[:, t, :].bitcast(F32R),
                            m3[:, t * 128:(t + 1) * 128],
                            idnr[:])
                    xo = work.tile([128, NTILE, 128], F32, tag="xo")
                    nc.vector.tensor_tensor(xo[:], em[:], he_nat[:], ALU.add)
                    for t in range(NTILE):
                        _ln_rows(nc, small, xo[:, t, :], eps_t[:], gb=gb1, bb=bb1)
                    nc.sync.dma_start(
                        out[e0:e0 + CH, :].rearrange(
                            "(t p) c -> p t c", p=128),
                        xo[:])

        if not eph:
            # ---- node-phase tail: dh, LN1, FFN, LN2, mask_V ----
            with tc.tile_pool(name="psB", bufs=1, space="PSUM") as psB:
                NT = ROWS // 128
                dh_s = ones.tile([C, ROWS], F32, tag="dh_s")
                for m in range(ROWS // 512):
                    sl = slice(m * 512, (m + 1) * 512)
                    pd = psB.tile([128, 512], F32, tag="pd")
                    nc.tensor.matmul(pd[:], w3_s[:], s2_s[:, sl],
                                     start=True, stop=False)
                    nc.tensor.matmul(pd[:], b3_s[:], msum_s[:, sl],
                                     start=False, stop=True)
                    nc.scalar.activation(dh_s[:, sl], pd[:],
                                         AF.Copy, scale=1.0 / SCALE)
                x1 = ones.tile([128, NT, C], F32, tag="x1")
                for t in range(NT):
                    pe = psB.tile([128, 128], F32, tag="pe", bufs=1)
                    nc.tensor.transpose(pe[:],
                                        dh_s[:, t * 128:(t + 1) * 128],
                                        idn[:])
                    nc.vector.tensor_tensor(x1[:, t, :], pe[:],
                                            hV_s[:, t, :], ALU.add)
                    _ln_rows(nc, small, x1[:, t, :], eps_t[:], gb=gb1, bb=bb1)
                # FFN on x1 (channel-major x1T), then LN2 + mask_V
                wfi_s = ones.tile([C, 4 * C], F32R, tag="wfi_s")
                nc.gpsimd.dma_start(wfi_s[:], wfi[:])
                wfo_s = ones.tile([128, 4, C], F32R, tag="wfo_s")
                nc.gpsimd.dma_start(
                    wfo_s[:], wfo[:].rearrange("(j p) c -> p j c", p=128))
                bfi_s = ones.tile([128, 4], F32, tag="bfi_s")
                nc.sync.dma_start(bfi_s[:], bfi[:])
                bfo_s = ones.tile([C, 1], F32, tag="bfo_s")
                nc.sync.dma_start(bfo_s[:], bfo[:])
                x1T = ones.tile([C, ROWS], F32R, tag="x1T")
                for t in range(NT):
                    px = psB.tile([128, 128], F32, tag="px", bufs=1)
                    nc.tensor.transpose(px[:], x1[:, t, :], idn[:])
                    nc.scalar.copy(x1T[:, t * 128:(t + 1) * 128], px[:])
                hf = ones.tile([128, 4, ROWS], F32R, tag="hf")
                for m in range(ROWS // 512):
                    sl = slice(m * 512, (m + 1) * 512)
                    for j in range(4):
                        pf = psB.tile([128, 512], F32, tag="pf", bufs=2)
                        nc.tensor.matmul(
                            pf[:], wfi_s[:, j * 128:(j + 1) * 128],
                            x1T[:, sl], start=True, stop=True)
                        nc.scalar.activation(hf[:, j, sl], pf[:], act,
                                             bias=bfi_s[:, j:j + 1])
                    po = psB.tile([128, 512], F32, tag="po")
                    for j in range(4):
                        nc.tensor.matmul(po[:], wfo_s[:, j, :],
                                         hf[:, j, sl],
                                         start=(j == 0), stop=(j == 3))
                    nc.scalar.activation(dh_s[:, sl], po[:], AF.Identity,
                                         bias=bfo_s[:])
                mv_s = ones.tile([128, NT], F32, tag="mv_s")
                nc.sync.dma_start(mv_s[:], maskv[:])
                for t in range(NT):
                    pe2 = psB.tile([128, 128], F32, tag="pe2", bufs=1)
                    nc.tensor.transpose(pe2[:],
                                        dh_s[:, t * 128:(t + 1) * 128],
                                        idn[:])
                    nc.vector.tensor_tensor(x1[:, t, :], pe2[:],
                                            x1[:, t, :], ALU.add)
                    _ln_rows(nc, small, x1[:, t, :], eps_t[:], gb=gb2, bb=bb2,
                             rowscale=mv_s[:, t:t + 1])
                nc.sync.dma_start(
                    out[:].rearrange("(t p) c -> p t c", p=128), x1[:])

    nc.compile()
    return nc


# ---------------------------------------------------------------------------
# host driver
# ---------------------------------------------------------------------------

def _np(x, dt=np.float32):
    return np.ascontiguousarray(np.asarray(x), dtype=dt)


def _shard(cid):
    return cid // 2, (cid % 2) * ROWS


def prep_phase1_inputs(h_V, h_E, E_idx, mask_V, mask_att, W, trivial_matt):
    W1a, W1c = W["W1"][:C], W["W1"][2 * C:]
    U1 = h_V @ W1c + W["b1"]
    U1tab = [_hi_lo_pack(U1[b]) for b in range(B)]
    ident = np.eye(128, dtype=np.float32)
    gbeta1 = np.concatenate([
        np.tile(W["g1"], (128, 1)), np.tile(W["beta1"], (128, 1)),
        np.tile(W["g2"], (128, 1)), np.tile(W["beta2"], (128, 1))], axis=1)
    trivial_gb = (np.all(W["g1"] == 1) and np.all(W["beta1"] == 0)
                  and np.all(W["g2"] == 1) and np.all(W["beta2"] == 0))
    maps = []
    for cid in range(NCORES):
        b, n0 = _shard(cid)
        rows = slice(n0, n0 + ROWS)
        m = {
            "hE": np.ascontiguousarray(h_E[b, rows].reshape(EDG, C)),
            "idx": _wrap_idx(E_idx[b, rows].reshape(-1)),
            "utab": U1tab[b],
            "pT": np.ascontiguousarray((h_V[b, rows] @ W1a).T),
            "wb": np.ascontiguousarray(W["W1"][C:2 * C]),
            "w2": W["W2"], "w3": W["W3"],
            "b2v": W["b2"].reshape(C, 1),
            "ident": ident, "identb": _bf(ident),
            "hV": np.ascontiguousarray(h_V[b, rows]),
            "b3v": W["b3"].reshape(1, C),
            "msum": np.ascontiguousarray(
                mask_att[b, rows].sum(-1, dtype=np.float32).reshape(1, ROWS)),
            "wfi": W["Wff_in"], "wfo": W["Wff_out"],
            "bfi": np.ascontiguousarray(W["bff_in"].reshape(4, 128).T),
            "bfo": W["bff_out"].reshape(C, 1),
            "maskv": np.ascontiguousarray(
                mask_V[b, rows].reshape(ROWS // 128, 128).T),
        }
        if not trivial_gb:
            m["gbeta"] = gbeta1
        if not trivial_matt:
            m["matt"] = np.ascontiguousarray(mask_att[b, rows].reshape(1, EDG))
        maps.append(m)
    return maps, trivial_gb


def prep_phase2_inputs(hV_new, in1, W):
    W11a, W11c = W["W11"][:C], W["W11"][2 * C:]
    U11 = hV_new @ W11c + W["b11"]
    U11tab = [_hi_lo_pack(U11[b]) for b in range(B)]
    ident = np.eye(128, dtype=np.float32)
    gbeta3 = np.concatenate([
        np.tile(W["g3"], (128, 1)), np.tile(W["beta3"], (128, 1))], axis=1)
    trivial_gb = np.all(W["g3"] == 1) and np.all(W["beta3"] == 0)
    maps = []
    for cid in range(NCORES):
        b, n0 = _shard(cid)
        rows = slice(n0, n0 + ROWS)
        m = {
            "hE": in1[cid]["hE"],
            "idx": in1[cid]["idx"],
            "utab": U11tab[b],
            "pT": np.ascontiguousarray((hV_new[b, rows] @ W11a).T),
            "wb": np.ascontiguousarray(W["W11"][C:2 * C]),
            "w2": W["W12"], "w3": W["W13"],
            "b2v": W["b12"].reshape(C, 1),
            "b3v": W["b13"].reshape(C, 1),
            "ident": ident, "identb": _bf(ident),
        }
        if not trivial_gb:
            m["gbeta"] = gbeta3
        maps.append(m)
    return maps, trivial_gb


TRACE = False
TIMING_ITERS = 0
LAST = {}


def _run_pjrt(nc, in_maps, n_cores, timing_iters=0):
    """run_bass_via_pjrt's multi-core path, without output donation, plus
    optional repeated timed executions (device-resident inputs)."""
    import time as _time
    import jax
    from jax.sharding import Mesh, PartitionSpec, NamedSharding
    from concourse import bass2jax as b2j
    from concourse import mybir as _mb

    b2j.install_neuronx_cc_hook()
    shard_map = b2j.shard_map

    partition_name = (nc.partition_id_tensor.name
                      if nc.partition_id_tensor else None)
    in_names, out_names, out_avals, zero_outs = [], [], [], []
    for alloc in nc.m.functions[0].allocations:
        if not isinstance(alloc, _mb.MemoryLocationSet):
            continue
        name = alloc.memorylocations[0].name
        if alloc.kind == "ExternalInput":
            if name != partition_name:
                in_names.append(name)
        elif alloc.kind == "ExternalOutput":
            out_names.append(name)
            shape = tuple(alloc.tensor_shape)
            dtype = _mb.dt.np(alloc.dtype)
            out_avals.append(jax.core.ShapedArray(shape, dtype))
            zero_outs.append(np.zeros(shape, dtype))
    n_params = len(in_names)
    all_in_names = in_names + out_names + (
        [partition_name] if partition_name else [])

    def _body(*args):
        operands = list(args)
        if partition_name is not None:
            operands.append(b2j.partition_id_tensor())
        return tuple(b2j._bass_exec_p.bind(
            *operands,
            out_avals=tuple(out_avals),
            in_names=tuple(all_in_names),
            out_names=tuple(out_names),
            lowering_input_output_aliases=(),
            sim_require_finite=True,
            sim_require_nnan=True,
            nc=nc,
        ))

    devices = jax.devices()[:n_cores]
    mesh = Mesh(np.asarray(devices), ("core",))
    nsh = NamedSharding(mesh, PartitionSpec("core"))
    in_specs = (PartitionSpec("core"),) * (n_params + len(out_names))
    out_specs = (PartitionSpec("core"),) * len(out_names)
    sharded = jax.jit(shard_map(
        _body, mesh=mesh, in_specs=in_specs, out_specs=out_specs,
        check_rep=False), keep_unused=True)
    concat_in = [
        np.concatenate([np.asarray(in_maps[c][in_names[i]])
                        for c in range(n_cores)], axis=0)
        for i in range(n_params)
    ]
    concat_zeros = [
        np.zeros((n_cores * z.shape[0], *z.shape[1:]), z.dtype)
        for z in zero_outs
    ]
    dev_in = [jax.device_put(a, nsh) for a in concat_in + concat_zeros]
    jax.block_until_ready(dev_in)
    out_arrs = sharded(*dev_in)
    jax.block_until_ready(out_arrs)
    times = []
    for _ in range(timing_iters):
        t0 = _time.perf_counter()
        o = sharded(*dev_in)
        jax.block_until_ready(o)
        times.append((_time.perf_counter() - t0) * 1e9)
    results = [
        {name: np.asarray(out_arrs[i]).reshape(
            n_cores, *out_avals[i].shape)[c]
         for i, name in enumerate(out_names)}
        for c in range(n_cores)
    ]
    return results, (min(times) if times else None)


def kernel(**inputs):
    h_V = _np(inputs["h_V"])
    h_E = _np(inputs["h_E"])
    E_idx = np.asarray(inputs["E_idx"]).astype(np.int64)
    mask_V = _np(inputs["mask_V"])
    mask_att = _np(inputs["mask_attend"])
    W = {k: _np(v) for k, v in inputs.items()
         if k not in ("h_V", "h_E", "E_idx", "mask_V", "mask_attend")}
    trivial_matt = bool(np.all(mask_att == 1))

    in1, tgb1 = prep_phase1_inputs(h_V, h_E, E_idx, mask_V, mask_att, W,
                                   trivial_matt)
    nc1 = _build_core(False, trivial_gb=tgb1, trivial_mask_att=trivial_matt)
    res1, t1 = _run_pjrt(nc1, in1, NCORES, timing_iters=TIMING_ITERS)
    LAST["phase1_ns"] = t1
    hV_new = np.empty_like(h_V)
    for cid in range(NCORES):
        b, n0 = _shard(cid)
        hV_new[b, n0:n0 + ROWS] = res1[cid]["out"]

    in2, tgb3 = prep_phase2_inputs(hV_new, in1, W)
    nc2 = _build_core(True, trivial_gb=tgb3, trivial_mask_att=True)
    res2, t2 = _run_pjrt(nc2, in2, NCORES, timing_iters=TIMING_ITERS)
    LAST["phase2_ns"] = t2
    hE_new = np.empty_like(h_E)
    for cid in range(NCORES):
        b, n0 = _shard(cid)
        hE_new[b, n0:n0 + ROWS] = res2[cid]["out"].reshape(ROWS, K, C)

    return hV_new, hE_new
